# revision 3
# baseline (speedup 1.0000x reference)
"""Multi-head attention Trainium2 kernel v2 (B=4, T=1024, C=1024, H=16, D=64).

Sharding over 8 NeuronCores: core c handles batch b = c//2 and head group
g = c%2 (heads [8g, 8g+8)).  Each core computes a partial out-projection
(its 8 heads' contribution, [T, C] bf16); the host sums the two partials per
batch and adds b_eff = b_out + b_v @ W_out (V-bias folded in).

v2 vs v1: all-bf16 matmul operands (no fp32r narrow-width penalty, half the
DMA bytes), PE clock warmup during the input DMA window, causal mask applied
additively in PSUM via a PE matmul (identity x mask) instead of Pool
multiplies, shorter softmax-normalize chain, interleaved qkt/attention/
out-projection emission order to keep PE fed.

Math (per core, bf16 matmuls, fp32 PSUM accumulation):
  XT = x[b].T (host, bf16 [C, T])
  QT/KT[f, t] = Wqk[:, f].T @ XT     (pair-stacked [128, T], Q*0.125 folded
                                      into W/b on host, bias via ACT copy)
  V[t, f]     = XT-chunk.T @ Wv      (ones col appended -> row sums)
  S^T[k, q]   = KT-slice.T @ QT-slice  (causal blocks; diagonal gets
                                        -1e9 mask added via ident@tri matmul)
  P           = exp(S^T)             (ACT, valid region, bf16 out)
  vals^T/s    = [V | 1].T @ P        (s = denominator in row 64)
  out[q, c]   = vals^T.T @ Wout-slice  (bf16 partial, host adds bias)
"""

import os
import numpy as np
import ml_dtypes

import concourse.bass as bass
import concourse.mybir as mybir
import concourse.tile as tile
from concourse import bacc
from concourse.bass_utils import run_bass_kernel_spmd

B, T, C, H, D = 4, 1024, 1024, 16, 64
P = 128            # partitions
HPC = 8            # heads per core
PAIRS = 4          # head pairs per core
KI = C // P        # 8 contraction tiles
QC = 512           # q-chunk (PSUM bank free size, fp32)
NQC = T // QC      # 2 q-chunks
F32 = mybir.dt.float32
F32R = mybir.dt.float32r
BF16 = mybir.dt.bfloat16
AF = mybir.ActivationFunctionType
ALU = mybir.AluOpType
BF = ml_dtypes.bfloat16

_CACHE = {}


def _build_nc():
    nc = bacc.Bacc(None, target_bir_lowering=False)

    xT = nc.dram_tensor("xT", [C, T], BF16, kind="ExternalInput")
    # wqk01: slots 0,1 (pair 0 Q,K); wqk27: slots 2..7 in one tensor
    wqk01 = nc.dram_tensor("wqk01", [2, P, KI, P], BF16, kind="ExternalInput")
    wqk27 = nc.dram_tensor("wqk27", [6, P, KI, P], BF16, kind="ExternalInput")
    wv = nc.dram_tensor("wv", [P, KI, HPC * D], BF16, kind="ExternalInput")
    wout = nc.dram_tensor("wout", [P, PAIRS, C], BF16, kind="ExternalInput")
    bqk = nc.dram_tensor("bqk", [P, 8], F32, kind="ExternalInput")
    # tri_add[k, q] = 0 if k <= q else -1e9 (additive causal mask block)
    tri_add = nc.dram_tensor("tri_add", [P, P], BF16, kind="ExternalInput")
    ident = nc.dram_tensor("ident", [P, P], BF16, kind="ExternalInput")
    # e2[s, m]: row 0 = 1 for m < 64, row 32 = 1 for m >= 64, else 0
    # (partition-broadcast matmul; s-rows live at partitions 0 and 32
    # because cross-partition DVE copies need offsets equal mod 32)
    e2 = nc.dram_tensor("e2", [33, P], BF16, kind="ExternalInput")
    out = nc.dram_tensor("out", [T, C], BF16, kind="ExternalOutput")

    xT_r = xT.rearrange("(ko p) t -> p ko t", p=P)

    with tile.TileContext(nc) as tc:
        with (
            tc.tile_pool(name="consts", bufs=1) as consts,
            tc.tile_pool(name="xt", bufs=8) as xt_pool,
            tc.tile_pool(name="wqk_p", bufs=4) as wqk_pool,
            tc.tile_pool(name="qkt", bufs=8) as qkt_pool,
            tc.tile_pool(name="vsb", bufs=8) as v_pool,
            tc.tile_pool(name="probs", bufs=12) as p_pool,
            tc.tile_pool(name="vals", bufs=8) as vals_pool,
            tc.tile_pool(name="smal", bufs=8) as small_pool,
            tc.tile_pool(name="outs", bufs=8) as out_pool,
        ):
            # ---- tiny consts on Pool SWDGE queue (head of queue) ----
            bqk_sb = consts.tile([P, 8], F32)
            nc.gpsimd.dma_start(bqk_sb, bqk[:, :])
            ident_sb = consts.tile([P, P], BF16)
            nc.gpsimd.dma_start(ident_sb, ident[:, :])
            tri_sb = consts.tile([P, P], BF16)
            nc.gpsimd.dma_start(tri_sb, tri_add[:, :])
            e2_sb = consts.tile([33, P], BF16)
            nc.gpsimd.dma_start(e2_sb, e2[:, :])

            # ---- warmup operands: memset then ~150 tiny matmuls to ramp
            # the PE clock while input DMAs are in flight ----
            warm_sb = consts.tile([16, 32], BF16)
            nc.vector.memset(warm_sb, 0.125)

            # ---- input DMAs, ordered by first-use time ----
            # The HWDGE generator is a serial mutex (~630ns per DMA) and the
            # DMA engines are one shared 360GB/s pipe, so queue order IS
            # arrival order.  First qkt slot consumes ki 0..7 in order:
            # alternate xt tiles across SP/ACT so they land in ki order.
            # xt as per-ki TILES (tile-granular deps: one big tile would
            # gate the first matmul on all 8 DMAs).
            w_sb = {}
            w_sb[0] = wqk_pool.tile([P, KI, P], BF16, tag="wqk", name="wqk0")
            nc.sync.dma_start(w_sb[0], wqk01[0])
            xt_sb = []
            for ki in range(KI):
                t_ = xt_pool.tile([P, T], BF16, tag="xt", name=f"xt{ki}")
                xt_sb.append(t_)
            for ki in range(0, KI, 2):
                nc.sync.dma_start(xt_sb[ki], xT_r[:, ki, :])
            for ki in range(1, KI, 2):
                nc.scalar.dma_start(xt_sb[ki], xT_r[:, ki, :])
            # wqk1 (K slot of pair 0) on SP behind the xt evens: lands
            # ~7.5us, right as slot q drains.
            w_sb[1] = wqk_pool.tile([P, KI, P], BF16, tag="wqk", name="wqk1")
            nc.sync.dma_start(w_sb[1], wqk01[1])
            # Pool SWDGE: wv in 4 two-ki tiles (V's ki loop chases their
            # arrival), then wqk 2..7 (needed from ~25us), wout last.
            wv_sb = []
            for kc in range(4):
                t_ = v_pool.tile([P, 2, HPC * D], BF16, tag="wv",
                                 name=f"wv{kc}")
                wv_sb.append(t_)
                nc.gpsimd.dma_start(t_, wv[:, 2 * kc : 2 * kc + 2, :])
            w27_sb = consts.tile([P, 6, KI, P], BF16, name="wqk27")
            w27_r = wqk27.rearrange("s p ko f -> p s ko f")
            for slot in range(2, 8):
                nc.gpsimd.dma_start(w27_sb[:, slot - 2], w27_r[:, slot - 2])
                w_sb[slot] = w27_sb[:, slot - 2]
            wout_sb = consts.tile([P, PAIRS, C], BF16)
            nc.gpsimd.dma_start(wout_sb, wout[:, :, :])

            # ---- PSUM pools (LIFO: qkv_ps opened last, closed first) ----
            s_ps_ctx = tc.tile_pool(name="s_ps", bufs=2, space="PSUM")
            s_ps = s_ps_ctx.__enter__()
            v_ps_ctx = tc.tile_pool(name="v_ps", bufs=2, space="PSUM")
            v_ps = v_ps_ctx.__enter__()
            qkv_ps_ctx = tc.tile_pool(name="qkv_ps", bufs=2, space="PSUM")
            qkv_ps = qkv_ps_ctx.__enter__()

            # warmup matmuls: [16,16]x[16,16] -> scratch psum, no deps
            # beyond the memset.  ~250 x ~13ns spans the 3us p-state ramp
            # so the first real matmul runs at full clock.
            warm_ps = qkv_ps.tile([16, 16], F32, tag="qkv", name="warm")
            for _ in range(0 if os.environ.get("V2_NO_WARM") else 250):
                nc.tensor.matmul(
                    warm_ps, warm_sb[:, 0:16], warm_sb[:, 16:32],
                    start=True, stop=True,
                )

            qt_sb = {}
            kt_sb = {}

            def emit_qkt(pair, kind):
                """QT/KT pair-stacked [128, T] bf16; ki-outer so PE chases
                the xt DMA arrival order on the first slot.  On that first
                slot, small warmup batches between ki steps keep the PE
                clock ramping through the DMA-feed gaps."""
                slot = 2 * pair + kind
                dst = qkt_pool.tile(
                    [P, T], BF16, tag="qkt", name=f"{'qk'[kind]}t{pair}"
                )
                ps = {}
                for qc in range(NQC):
                    ps[qc] = qkv_ps.tile([P, QC], F32, tag="qkv", name=f"qkvps{qc}")
                for ki in range(KI):
                    for qc in range(NQC):
                        nc.tensor.matmul(
                            ps[qc],
                            w_sb[slot][:, ki, :],
                            xt_sb[ki][:, qc * QC : (qc + 1) * QC],
                            start=(ki == 0),
                            stop=(ki == KI - 1),
                        )
                for qc in range(NQC):
                    nc.scalar.activation(
                        dst[:, qc * QC : (qc + 1) * QC],
                        ps[qc],
                        AF.Identity,
                        bias=bqk_sb[:, slot : slot + 1],
                    )
                return dst

            v_sb = []

            def emit_v(ti):
                vt = v_pool.tile([P, HPC, D + 1], BF16, tag="v_sb", name=f"v{ti}")
                v_sb.append(vt)
                ps = qkv_ps.tile([P, QC], F32, tag="qkv")
                for ki in range(KI):
                    nc.tensor.matmul(
                        ps,
                        xt_sb[ki][:, ti * P : (ti + 1) * P],
                        wv_sb[ki // 2][:, ki % 2, :],
                        start=(ki == 0),
                        stop=(ki == KI - 1),
                    )
                nc.vector.tensor_copy(
                    vt[:, :, 0:D], ps.rearrange("p (h d) -> p h d", h=HPC)
                )
                nc.vector.memset(vt[:, :, D : D + 1], 1.0)

            # ---- attention ----
            vals_sb = {}   # (pair, qc) -> [P, QC] bf16

            def emit_scores(pair, qc):
                """Score matmuls + exp for both heads of the pair over
                q-chunk qc; returns the exp'd probability tiles."""
                qt = qt_sb[pair]
                kt = kt_sb[pair]
                n_kt = 4 * (qc + 1)
                p_tiles = []
                for kj in range(n_kt):
                    j0 = kj - 4 * qc
                    q_lo = max(j0, 0) * P
                    pt = p_pool.tile([P, 2, QC], BF16, tag="probs")
                    p_tiles.append((pt, q_lo))
                    sps = s_ps.tile([P, 2, QC], F32, tag="s", name="sps")
                    diag = j0 >= 0 and not os.environ.get("V2_NO_MASKMM")
                    for hl in range(2):
                        d0 = D * hl
                        nc.tensor.matmul(
                            sps[:, hl, q_lo:QC],
                            kt[d0 : d0 + D, kj * P : (kj + 1) * P],
                            qt[d0 : d0 + D, qc * QC + q_lo : (qc + 1) * QC],
                            start=True,
                            stop=not diag,
                            skip_group_check=True,
                        )
                        if diag:
                            # additive -1e9 causal mask on the diagonal block
                            nc.tensor.matmul(
                                sps[:, hl, q_lo : q_lo + P],
                                ident_sb,
                                tri_sb,
                                start=False,
                                stop=True,
                                skip_group_check=True,
                            )
                    nc.scalar.activation(
                        pt[:, :, q_lo:QC], sps[:, :, q_lo:QC], AF.Exp
                    )
                return p_tiles

            def emit_attnv(pair, qc, p_tiles, pe_bcast=False):
                """attnV accumulation + softmax normalize.  pe_bcast:
                broadcast the reciprocal via a PE matmul instead of a DMA
                (shorter chain) - used for the chain-critical final pair.
                Returns a closure emitting the final normalize multiply."""
                n_kt = 4 * (qc + 1)
                key = (pair, qc)
                vals = vals_pool.tile([P, QC], BF16, tag="vals",
                                      name=f"vals{pair}_{qc}")
                vals_sb[key] = vals
                # vals_u: unnormalized vals evicted to SBUF right after each
                # head's accumulation -- frees the vps PSUM buffer in ~1.3us
                # instead of holding it through the whole normalize chain
                # (which serialized the next pair's attnV against it).
                vals_u = vals_pool.tile([P, QC], BF16, tag="valsu",
                                        name=f"valsu{pair}_{qc}")
                s2 = small_pool.tile([33, QC], F32, tag="s2")
                # rows 1..31 feed the broadcast matmul (x 0 weights);
                # clear them so recip never turns garbage into NaN
                nc.vector.memset(s2, 1.0)
                for hl in range(2):
                    h_abs = 2 * pair + hl
                    vps = v_ps.tile([P, QC], F32, tag="vps", name=f"vps{hl}")
                    for kj in range(n_kt):
                        pt, q_lo = p_tiles[kj]
                        nc.tensor.matmul(
                            vps[0 : D + 1, q_lo:QC],
                            v_sb[kj][:, h_abs, :],
                            pt[:, hl, q_lo:QC],
                            start=(kj == 0),
                            stop=(kj == n_kt - 1),
                            skip_group_check=True,
                        )
                    # s-row copy: partition 64 -> 0 / 32 (DVE cross-
                    # partition moves need offsets equal mod 32); then the
                    # unnormalized eviction releases vps.
                    nc.vector.tensor_copy(
                        s2[32 * hl : 32 * hl + 1, :], vps[D : D + 1, :]
                    )
                    nc.vector.tensor_copy(
                        vals_u[D * hl : D * (hl + 1), :], vps[0:D, :]
                    )
                # normalize in SBUF: recip over partitions 0..32 (covers
                # both s-rows; rows 1..31 garbage, never read), bf16 round,
                # broadcast to 128 partitions (DMA, or a PE matmul for the
                # chain-critical final pair), one bf16 multiply.
                # reciprocal, bf16 round, broadcast to 128 partitions via
                # DMA (PE matmul for the chain-critical final pair).  The
                # multiply is returned as a closure: inline it would park
                # the in-order DVE stream on the r_bc DMA round-trip.
                r2 = small_pool.tile([33, QC], F32, tag="r2")
                nc.vector.reciprocal_approx_fast(r2, s2)
                r2b = small_pool.tile([33, QC], BF16, tag="r2b")
                nc.vector.tensor_copy(r2b, r2)
                if pe_bcast:
                    rps = s_ps.tile([P, QC], F32, tag="s", name="rps")
                    nc.tensor.matmul(rps, e2_sb, r2b, start=True, stop=True)
                    r_bc = rps
                else:
                    r_bc = small_pool.tile([P, QC], BF16, tag="rbc")
                    for hl in range(2):
                        nc.sync.dma_start(
                            r_bc[D * hl : D * (hl + 1), :],
                            r2b[32 * hl : 32 * hl + 1, None, :]
                            .to_broadcast([1, D, QC]),
                        )

                def emit_mult():
                    nc.vector.tensor_tensor(vals, vals_u, r_bc, ALU.mult)
                return emit_mult

            o_ps_holder = {}

            def emit_out_group(qc, tsub, cc, last=False):
                """One out-projection group: [128 q, 512 c] accumulated
                over the 4 pairs, eviction to bf16 (Pool mid-kernel while
                ACT runs exps, ACT in the tail), SP store."""
                q0 = tsub * P
                o_ps = o_ps_holder["pool"]
                ops = o_ps.tile([P, QC], F32, tag="ops")
                for pair in range(PAIRS):
                    nc.tensor.matmul(
                        ops,
                        vals_sb[(pair, qc)][:, q0 : q0 + P],
                        wout_sb[:, pair, cc * QC : (cc + 1) * QC],
                        start=(pair == 0),
                        stop=(pair == PAIRS - 1),
                    )
                o_sb = out_pool.tile([P, QC], BF16, tag="o_sb")
                if last:
                    # final group: halve the evict+store so the kernel tail
                    # is one 256-col chunk, split across ACT and DVE
                    nc.scalar.activation(o_sb[:, 0:256], ops[:, 0:256], AF.Copy)
                    nc.vector.tensor_copy(o_sb[:, 256:512], ops[:, 256:512])
                    for h, eng in enumerate((nc.sync, nc.scalar)):
                        eng.dma_start(
                            out[qc * QC + q0 : qc * QC + q0 + P,
                                cc * QC + 256 * h : cc * QC + 256 * (h + 1)],
                            o_sb[:, 256 * h : 256 * (h + 1)],
                        )
                else:
                    nc.scalar.activation(o_sb, ops, AF.Copy)
                    nc.sync.dma_start(
                        out[qc * QC + q0 : qc * QC + q0 + P,
                            cc * QC : (cc + 1) * QC],
                        o_sb,
                    )

            def emit_attention(pair, qc, pe_bcast=False):
                return emit_attnv(pair, qc, emit_scores(pair, qc), pe_bcast)

            # ---- emission order ----
            qt_sb[0] = emit_qkt(0, 0)
            kt_sb[0] = emit_qkt(0, 1)
            for ti in range(T // P):
                emit_v(ti)
            qt_sb[1] = emit_qkt(1, 0)
            kt_sb[1] = emit_qkt(1, 1)
            mults_q1_0 = emit_attention(0, 1)
            qt_sb[2] = emit_qkt(2, 0)
            kt_sb[2] = emit_qkt(2, 1)
            mults_q1_1 = emit_attention(1, 1)
            qt_sb[3] = emit_qkt(3, 0)
            kt_sb[3] = emit_qkt(3, 1)
            mults_q1_2 = emit_attention(2, 1)
            mults_q1_3 = emit_attention(3, 1)
            # projections done: release qkv psum (2 banks) for out-proj
            qkv_ps_ctx.__exit__(None, None, None)
            o_ps_ctx = tc.tile_pool(name="o_ps", bufs=2, space="PSUM")
            o_ps_holder["pool"] = o_ps_ctx.__enter__()

            # qc0 pairs in S/S/A/A form: pair p+1's scores run while pair
            # p's exps trail, so the attnV never waits on a fresh exp.
            mults = {}
            mults[(0, 0)] = emit_attention(0, 0)
            mults[(1, 0)] = emit_attention(1, 0)
            # qc1 normalize mults: r_bc broadcasts have long since landed
            for m in (mults_q1_0, mults_q1_1, mults_q1_2, mults_q1_3):
                m()
            for tsub in range(2):
                for cc in range(2):
                    emit_out_group(1, tsub, cc)
            mults[(2, 0)] = emit_attention(2, 0)
            mults[(3, 0)] = emit_attention(3, 0, pe_bcast=True)
            # out(1) g5-8 here: ~3.4us of PE work with no dependency on
            # pair 3's normalize chain -- hides it completely.
            for tsub in range(2, 4):
                for cc in range(2):
                    emit_out_group(1, tsub, cc)
            for pair in range(PAIRS):
                mults[(pair, 0)]()
            for tsub in range(4):
                for cc in range(2):
                    emit_out_group(0, tsub, cc, last=(tsub == 3 and cc == 1))

            o_ps_ctx.__exit__(None, None, None)
            v_ps_ctx.__exit__(None, None, None)
            s_ps_ctx.__exit__(None, None, None)

    nc.compile()
    return nc


def _host_shards(x, mask, W_in, b_in, W_out, b_out):
    """Build the 8 per-core input maps (bf16, pre-shuffled layouts)."""
    del mask  # causal structure hardcoded (tri_add built locally)
    x = np.asarray(x, dtype=np.float32)
    W_in = np.asarray(W_in, dtype=np.float32)
    b_in = np.asarray(b_in, dtype=np.float32)
    W_out = np.asarray(W_out, dtype=np.float32)

    k = np.arange(P)
    tri_add = np.where(k[:, None] <= k[None, :], 0.0, -1e9).astype(BF)
    ident = np.eye(P, dtype=BF)
    e2 = np.zeros((33, P), dtype=BF)
    e2[0, 0:D] = 1.0
    e2[32, D:P] = 1.0
    xTs = [np.ascontiguousarray(x[b].T.astype(BF)) for b in range(B)]

    per_group = {}
    for g in range(2):
        wqk = np.empty((8, P, KI, P), dtype=BF)
        bqk = np.empty((P, 8), dtype=np.float32)
        wi_r = W_in.reshape(KI, P, 3 * C)
        for p in range(PAIRS):
            qcols = slice((8 * g + 2 * p) * D, (8 * g + 2 * p + 2) * D)
            kcols = slice(C + (8 * g + 2 * p) * D, C + (8 * g + 2 * p + 2) * D)
            # wqk[slot, p, ki, f] = W_in[ki*128+p, cols[f]] (x scale for Q)
            wqk[2 * p] = (wi_r[:, :, qcols] * 0.125).transpose(1, 0, 2).astype(BF)
            wqk[2 * p + 1] = wi_r[:, :, kcols].transpose(1, 0, 2).astype(BF)
            bqk[:, 2 * p] = b_in[qcols] * 0.125
            bqk[:, 2 * p + 1] = b_in[kcols]
        vcols = slice(2 * C + g * 512, 2 * C + (g + 1) * 512)
        wv = np.ascontiguousarray(
            wi_r[:, :, vcols].transpose(1, 0, 2).astype(BF)
        )
        # wout[p, po, c] = W_out[g*512 + po*128 + p, c]
        wo = W_out[g * 512 : (g + 1) * 512, :].reshape(PAIRS, P, C)
        wout = np.ascontiguousarray(wo.transpose(1, 0, 2).astype(BF))
        per_group[g] = dict(
            wqk01=np.ascontiguousarray(wqk[0:2]),
            wqk27=np.ascontiguousarray(wqk[2:8]),
            bqk=bqk, wv=wv, wout=wout,
            tri_add=tri_add, ident=ident, e2=e2,
        )

    in_maps = []
    for c in range(8):
        b, g = c // 2, c % 2
        m = dict(per_group[g])
        m["xT"] = xTs[b]
        in_maps.append(m)
    return in_maps


def run(inputs, trace=False):
    if "nc" not in _CACHE:
        _CACHE["nc"] = _build_nc()
    nc = _CACHE["nc"]
    in_maps = _host_shards(**inputs)
    res = run_bass_kernel_spmd(
        nc, in_maps, core_ids=list(range(8)), trace=trace,
        trace_cores=list(range(8)) if trace else None,
    )
    b_in = np.asarray(inputs["b_in"], dtype=np.float32)
    W_out = np.asarray(inputs["W_out"], dtype=np.float32)
    b_out = np.asarray(inputs["b_out"], dtype=np.float32)
    # V-bias folded into the output bias: b_eff = b_out + b_v @ W_out
    b_eff = b_out + b_in[2 * C :].astype(np.float32) @ W_out
    out = np.empty((B, T, C), dtype=np.float32)
    for b in range(B):
        out[b] = (
            np.asarray(res.results[2 * b]["out"], dtype=np.float32)
            + np.asarray(res.results[2 * b + 1]["out"], dtype=np.float32)
            + b_eff
        )
    return out, res


def kernel(**inputs) -> np.ndarray:
    out, _ = run(inputs, trace=False)
    return out


# revision 4
# speedup vs baseline: 1.0140x; 1.0140x over previous
"""Multi-head attention Trainium2 kernel v2 (B=4, T=1024, C=1024, H=16, D=64).

Sharding over 8 NeuronCores: core c handles batch b = c//2 and head group
g = c%2 (heads [8g, 8g+8)).  Each core computes a partial out-projection
(its 8 heads' contribution, [T, C] bf16); the host sums the two partials per
batch and adds b_eff = b_out + b_v @ W_out (V-bias folded in).

v2 vs v1: all-bf16 matmul operands (no fp32r narrow-width penalty, half the
DMA bytes), PE clock warmup during the input DMA window, causal mask applied
additively in PSUM via a PE matmul (identity x mask) instead of Pool
multiplies, shorter softmax-normalize chain, interleaved qkt/attention/
out-projection emission order to keep PE fed.

Math (per core, bf16 matmuls, fp32 PSUM accumulation):
  XT = x[b].T (host, bf16 [C, T])
  QT/KT[f, t] = Wqk[:, f].T @ XT     (pair-stacked [128, T], Q*0.125 folded
                                      into W/b on host, bias via ACT copy)
  V[t, f]     = XT-chunk.T @ Wv      (ones col appended -> row sums)
  S^T[k, q]   = KT-slice.T @ QT-slice  (causal blocks; diagonal gets
                                        -1e9 mask added via ident@tri matmul)
  P           = exp(S^T)             (ACT, valid region, bf16 out)
  vals^T/s    = [V | 1].T @ P        (s = denominator in row 64)
  out[q, c]   = vals^T.T @ Wout-slice  (bf16 partial, host adds bias)
"""

import os
import numpy as np
import ml_dtypes

import concourse.bass as bass
import concourse.mybir as mybir
import concourse.tile as tile
from concourse import bacc
from concourse.bass_utils import run_bass_kernel_spmd

B, T, C, H, D = 4, 1024, 1024, 16, 64
P = 128            # partitions
HPC = 8            # heads per core
PAIRS = 4          # head pairs per core
KI = C // P        # 8 contraction tiles
QC = 512           # q-chunk (PSUM bank free size, fp32)
NQC = T // QC      # 2 q-chunks
F32 = mybir.dt.float32
F32R = mybir.dt.float32r
BF16 = mybir.dt.bfloat16
AF = mybir.ActivationFunctionType
ALU = mybir.AluOpType
BF = ml_dtypes.bfloat16

_CACHE = {}


def _build_nc():
    nc = bacc.Bacc(None, target_bir_lowering=False)

    xT = nc.dram_tensor("xT", [C, T], BF16, kind="ExternalInput")
    # wqk01: slots 0,1 (pair 0 Q,K); wqk27: slots 2..7 in one tensor
    wqk01 = nc.dram_tensor("wqk01", [2, P, KI, P], BF16, kind="ExternalInput")
    wqk27 = nc.dram_tensor("wqk27", [6, P, KI, P], BF16, kind="ExternalInput")
    wv = nc.dram_tensor("wv", [P, KI, HPC * D], BF16, kind="ExternalInput")
    wout = nc.dram_tensor("wout", [P, PAIRS, C], BF16, kind="ExternalInput")
    bqk = nc.dram_tensor("bqk", [P, 8], F32, kind="ExternalInput")
    # tri_add[k, q] = 0 if k <= q else -1e9 (additive causal mask block)
    tri_add = nc.dram_tensor("tri_add", [P, P], BF16, kind="ExternalInput")
    ident = nc.dram_tensor("ident", [P, P], BF16, kind="ExternalInput")
    # e2[s, m]: row 0 = 1 for m < 64, row 32 = 1 for m >= 64, else 0
    # (partition-broadcast matmul; s-rows live at partitions 0 and 32
    # because cross-partition DVE copies need offsets equal mod 32)
    e2 = nc.dram_tensor("e2", [33, P], BF16, kind="ExternalInput")
    out = nc.dram_tensor("out", [T, C], BF16, kind="ExternalOutput")

    xT_r = xT.rearrange("(ko p) t -> p ko t", p=P)

    with tile.TileContext(nc) as tc:
        with (
            tc.tile_pool(name="consts", bufs=1) as consts,
            tc.tile_pool(name="xt", bufs=8) as xt_pool,
            tc.tile_pool(name="wqk_p", bufs=4) as wqk_pool,
            tc.tile_pool(name="qkt", bufs=8) as qkt_pool,
            tc.tile_pool(name="vsb", bufs=8) as v_pool,
            tc.tile_pool(name="probs", bufs=12) as p_pool,
            tc.tile_pool(name="vals", bufs=8) as vals_pool,
            tc.tile_pool(name="smal", bufs=8) as small_pool,
            tc.tile_pool(name="outs", bufs=8) as out_pool,
        ):
            # ---- tiny consts on Pool SWDGE queue (head of queue) ----
            bqk_sb = consts.tile([P, 8], F32)
            nc.gpsimd.dma_start(bqk_sb, bqk[:, :])
            ident_sb = consts.tile([P, P], BF16)
            nc.gpsimd.dma_start(ident_sb, ident[:, :])
            tri_sb = consts.tile([P, P], BF16)
            nc.gpsimd.dma_start(tri_sb, tri_add[:, :])
            e2_sb = consts.tile([33, P], BF16)
            nc.gpsimd.dma_start(e2_sb, e2[:, :])

            # ---- warmup operands: memset then ~150 tiny matmuls to ramp
            # the PE clock while input DMAs are in flight ----
            warm_sb = consts.tile([16, 32], BF16)
            nc.vector.memset(warm_sb, 0.125)

            # ---- input DMAs, ordered by first-use time ----
            # The HWDGE generator is a serial mutex (~630ns per DMA) and the
            # DMA engines are one shared 360GB/s pipe, so queue order IS
            # arrival order.  First qkt slot consumes ki 0..7 in order:
            # alternate xt tiles across SP/ACT so they land in ki order.
            # xt as per-ki TILES (tile-granular deps: one big tile would
            # gate the first matmul on all 8 DMAs).
            w_sb = {}
            w_sb[0] = wqk_pool.tile([P, KI, P], BF16, tag="wqk", name="wqk0")
            nc.sync.dma_start(w_sb[0], wqk01[0])
            xt_sb = []
            for ki in range(KI):
                t_ = xt_pool.tile([P, T], BF16, tag="xt", name=f"xt{ki}")
                xt_sb.append(t_)
            for ki in range(0, KI, 2):
                nc.sync.dma_start(xt_sb[ki], xT_r[:, ki, :])
            for ki in range(1, KI, 2):
                nc.scalar.dma_start(xt_sb[ki], xT_r[:, ki, :])
            # wqk1 (K slot of pair 0) on SP behind the xt evens: lands
            # ~7.5us, right as slot q drains.
            w_sb[1] = wqk_pool.tile([P, KI, P], BF16, tag="wqk", name="wqk1")
            nc.sync.dma_start(w_sb[1], wqk01[1])
            # Pool SWDGE: wv in 4 two-ki tiles (V's ki loop chases their
            # arrival), then wqk 2..7 (needed from ~25us), wout last.
            wv_sb = []
            for kc in range(4):
                t_ = v_pool.tile([P, 2, HPC * D], BF16, tag="wv",
                                 name=f"wv{kc}")
                wv_sb.append(t_)
                nc.gpsimd.dma_start(t_, wv[:, 2 * kc : 2 * kc + 2, :])
            w27_sb = consts.tile([P, 6, KI, P], BF16, name="wqk27")
            w27_r = wqk27.rearrange("s p ko f -> p s ko f")
            for slot in range(2, 8):
                nc.gpsimd.dma_start(w27_sb[:, slot - 2], w27_r[:, slot - 2])
                w_sb[slot] = w27_sb[:, slot - 2]
            wout_sb = consts.tile([P, PAIRS, C], BF16)
            nc.gpsimd.dma_start(wout_sb, wout[:, :, :])

            # ---- PSUM pools (LIFO: qkv_ps opened last, closed first) ----
            s_ps_ctx = tc.tile_pool(name="s_ps", bufs=2, space="PSUM")
            s_ps = s_ps_ctx.__enter__()
            v_ps_ctx = tc.tile_pool(name="v_ps", bufs=2, space="PSUM")
            v_ps = v_ps_ctx.__enter__()
            qkv_ps_ctx = tc.tile_pool(name="qkv_ps", bufs=2, space="PSUM")
            qkv_ps = qkv_ps_ctx.__enter__()

            # warmup matmuls: [16,16]x[16,16] -> scratch psum, no deps
            # beyond the memset.  ~250 x ~13ns spans the 3us p-state ramp
            # so the first real matmul runs at full clock.
            warm_ps = qkv_ps.tile([16, 16], F32, tag="qkv", name="warm")
            for _ in range(0 if os.environ.get("V2_NO_WARM") else 250):
                nc.tensor.matmul(
                    warm_ps, warm_sb[:, 0:16], warm_sb[:, 16:32],
                    start=True, stop=True,
                )

            qt_sb = {}
            kt_sb = {}

            def emit_qkt(pair, kind, step_cb=None):
                """QT/KT pair-stacked [128, T] bf16; ki-outer so PE chases
                the xt DMA arrival order on the first slot.  step_cb(ki)
                lets the caller interleave independent work (V tiles) into
                the DMA-feed gaps of that first slot."""
                slot = 2 * pair + kind
                dst = qkt_pool.tile(
                    [P, T], BF16, tag="qkt", name=f"{'qk'[kind]}t{pair}"
                )
                ps = {}
                for qc in range(NQC):
                    ps[qc] = qkv_ps.tile([P, QC], F32, tag="qkv", name=f"qkvps{qc}")
                for ki in range(KI):
                    for qc in range(NQC):
                        nc.tensor.matmul(
                            ps[qc],
                            w_sb[slot][:, ki, :],
                            xt_sb[ki][:, qc * QC : (qc + 1) * QC],
                            start=(ki == 0),
                            stop=(ki == KI - 1),
                        )
                    if step_cb is not None:
                        step_cb(ki)
                for qc in range(NQC):
                    nc.scalar.activation(
                        dst[:, qc * QC : (qc + 1) * QC],
                        ps[qc],
                        AF.Identity,
                        bias=bqk_sb[:, slot : slot + 1],
                    )
                return dst

            v_sb = []

            def emit_v_interleaved():
                """V tiles 0 and 1 built inside slot q's feed gaps: their
                matmuls trail the slot's ki steps by 4 (so xt and the wv
                chunks have landed), turning DMA-wait idle into work.
                PSUM from the v_ps ring (idle until attention)."""
                vts, pss = [], []
                for ti in range(2):
                    vt = v_pool.tile([P, HPC, D + 1], BF16, tag="v_sb",
                                     name=f"v{ti}")
                    v_sb.append(vt)
                    vts.append(vt)
                    pss.append(v_ps.tile([P, QC], F32, tag="vps",
                                         name=f"vips{ti}"))

                def step(ki):
                    for kv in ([ki - 4] if ki >= 4 else []) + (
                        [ki - 3, ki - 2, ki - 1, ki] if ki == KI - 1 else []
                    ):
                        for ti in range(2):
                            nc.tensor.matmul(
                                pss[ti],
                                xt_sb[kv][:, ti * P : (ti + 1) * P],
                                wv_sb[kv // 2][:, kv % 2, :],
                                start=(kv == 0),
                                stop=(kv == KI - 1),
                            )

                def finish():
                    for ti in range(2):
                        nc.vector.tensor_copy(
                            vts[ti][:, :, 0:D],
                            pss[ti].rearrange("p (h d) -> p h d", h=HPC),
                        )
                        nc.vector.memset(vts[ti][:, :, D : D + 1], 1.0)

                return step, finish

            def emit_v(ti):
                vt = v_pool.tile([P, HPC, D + 1], BF16, tag="v_sb", name=f"v{ti}")
                v_sb.append(vt)
                ps = qkv_ps.tile([P, QC], F32, tag="qkv")
                for ki in range(KI):
                    nc.tensor.matmul(
                        ps,
                        xt_sb[ki][:, ti * P : (ti + 1) * P],
                        wv_sb[ki // 2][:, ki % 2, :],
                        start=(ki == 0),
                        stop=(ki == KI - 1),
                    )
                nc.vector.tensor_copy(
                    vt[:, :, 0:D], ps.rearrange("p (h d) -> p h d", h=HPC)
                )
                nc.vector.memset(vt[:, :, D : D + 1], 1.0)

            # ---- attention ----
            vals_sb = {}   # (pair, qc) -> [P, QC] bf16

            def emit_scores(pair, qc):
                """Score matmuls + exp for both heads of the pair over
                q-chunk qc; returns the exp'd probability tiles."""
                qt = qt_sb[pair]
                kt = kt_sb[pair]
                n_kt = 4 * (qc + 1)
                p_tiles = []
                for kj in range(n_kt):
                    j0 = kj - 4 * qc
                    q_lo = max(j0, 0) * P
                    pt = p_pool.tile([P, 2, QC], BF16, tag="probs")
                    p_tiles.append((pt, q_lo))
                    sps = s_ps.tile([P, 2, QC], F32, tag="s", name="sps")
                    diag = j0 >= 0 and not os.environ.get("V2_NO_MASKMM")
                    for hl in range(2):
                        d0 = D * hl
                        nc.tensor.matmul(
                            sps[:, hl, q_lo:QC],
                            kt[d0 : d0 + D, kj * P : (kj + 1) * P],
                            qt[d0 : d0 + D, qc * QC + q_lo : (qc + 1) * QC],
                            start=True,
                            stop=not diag,
                            skip_group_check=True,
                        )
                        if diag:
                            # additive -1e9 causal mask on the diagonal block
                            nc.tensor.matmul(
                                sps[:, hl, q_lo : q_lo + P],
                                ident_sb,
                                tri_sb,
                                start=False,
                                stop=True,
                                skip_group_check=True,
                            )
                    nc.scalar.activation(
                        pt[:, :, q_lo:QC], sps[:, :, q_lo:QC], AF.Exp
                    )
                return p_tiles

            def emit_attnv(pair, qc, p_tiles, pe_bcast=False):
                """attnV accumulation + softmax normalize.  pe_bcast:
                broadcast the reciprocal via a PE matmul instead of a DMA
                (shorter chain) - used for the chain-critical final pair.
                Returns a closure emitting the final normalize multiply."""
                n_kt = 4 * (qc + 1)
                key = (pair, qc)
                vals = vals_pool.tile([P, QC], BF16, tag="vals",
                                      name=f"vals{pair}_{qc}")
                vals_sb[key] = vals
                # vals_u: unnormalized vals evicted to SBUF right after each
                # head's accumulation -- frees the vps PSUM buffer in ~1.3us
                # instead of holding it through the whole normalize chain
                # (which serialized the next pair's attnV against it).
                vals_u = vals_pool.tile([P, QC], BF16, tag="valsu",
                                        name=f"valsu{pair}_{qc}")
                s2 = small_pool.tile([33, QC], F32, tag="s2")
                # rows 1..31 feed the broadcast matmul (x 0 weights);
                # clear them so recip never turns garbage into NaN
                nc.vector.memset(s2, 1.0)
                for hl in range(2):
                    h_abs = 2 * pair + hl
                    vps = v_ps.tile([P, QC], F32, tag="vps", name=f"vps{hl}")
                    for kj in range(n_kt):
                        pt, q_lo = p_tiles[kj]
                        nc.tensor.matmul(
                            vps[0 : D + 1, q_lo:QC],
                            v_sb[kj][:, h_abs, :],
                            pt[:, hl, q_lo:QC],
                            start=(kj == 0),
                            stop=(kj == n_kt - 1),
                            skip_group_check=True,
                        )
                    # s-row copy: partition 64 -> 0 / 32 (DVE cross-
                    # partition moves need offsets equal mod 32); then the
                    # unnormalized eviction releases vps.
                    nc.vector.tensor_copy(
                        s2[32 * hl : 32 * hl + 1, :], vps[D : D + 1, :]
                    )
                    nc.vector.tensor_copy(
                        vals_u[D * hl : D * (hl + 1), :], vps[0:D, :]
                    )
                # normalize in SBUF: recip over partitions 0..32 (covers
                # both s-rows; rows 1..31 garbage, never read), bf16 round,
                # broadcast to 128 partitions (DMA, or a PE matmul for the
                # chain-critical final pair), one bf16 multiply.
                # reciprocal, bf16 round, broadcast to 128 partitions via
                # DMA (PE matmul for the chain-critical final pair).  The
                # multiply is returned as a closure: inline it would park
                # the in-order DVE stream on the r_bc DMA round-trip.
                r2 = small_pool.tile([33, QC], F32, tag="r2")
                nc.vector.reciprocal_approx_fast(r2, s2)
                r2b = small_pool.tile([33, QC], BF16, tag="r2b")
                nc.vector.tensor_copy(r2b, r2)
                if pe_bcast:
                    rps = s_ps.tile([P, QC], F32, tag="s", name="rps")
                    nc.tensor.matmul(rps, e2_sb, r2b, start=True, stop=True)
                    r_bc = rps
                else:
                    r_bc = small_pool.tile([P, QC], BF16, tag="rbc")
                    for hl in range(2):
                        nc.sync.dma_start(
                            r_bc[D * hl : D * (hl + 1), :],
                            r2b[32 * hl : 32 * hl + 1, None, :]
                            .to_broadcast([1, D, QC]),
                        )

                def emit_mult():
                    nc.vector.tensor_tensor(vals, vals_u, r_bc, ALU.mult)
                return emit_mult

            o_ps_holder = {}

            def emit_out_group(qc, tsub, cc, last=False):
                """One out-projection group: [128 q, 512 c] accumulated
                over the 4 pairs, eviction to bf16 (Pool mid-kernel while
                ACT runs exps, ACT in the tail), SP store."""
                q0 = tsub * P
                o_ps = o_ps_holder["pool"]
                ops = o_ps.tile([P, QC], F32, tag="ops")
                for pair in range(PAIRS):
                    nc.tensor.matmul(
                        ops,
                        vals_sb[(pair, qc)][:, q0 : q0 + P],
                        wout_sb[:, pair, cc * QC : (cc + 1) * QC],
                        start=(pair == 0),
                        stop=(pair == PAIRS - 1),
                    )
                o_sb = out_pool.tile([P, QC], BF16, tag="o_sb")
                if last:
                    # final group: halve the evict+store so the kernel tail
                    # is one 256-col chunk, split across ACT and DVE
                    nc.scalar.activation(o_sb[:, 0:256], ops[:, 0:256], AF.Copy)
                    nc.vector.tensor_copy(o_sb[:, 256:512], ops[:, 256:512])
                    for h, eng in enumerate((nc.sync, nc.scalar)):
                        eng.dma_start(
                            out[qc * QC + q0 : qc * QC + q0 + P,
                                cc * QC + 256 * h : cc * QC + 256 * (h + 1)],
                            o_sb[:, 256 * h : 256 * (h + 1)],
                        )
                else:
                    nc.scalar.activation(o_sb, ops, AF.Copy)
                    nc.sync.dma_start(
                        out[qc * QC + q0 : qc * QC + q0 + P,
                            cc * QC : (cc + 1) * QC],
                        o_sb,
                    )

            def emit_attention(pair, qc, pe_bcast=False):
                return emit_attnv(pair, qc, emit_scores(pair, qc), pe_bcast)

            # ---- emission order ----
            qt_sb[0] = emit_qkt(0, 0)
            kt_sb[0] = emit_qkt(0, 1)
            for ti in range(T // P):
                emit_v(ti)
            qt_sb[1] = emit_qkt(1, 0)
            kt_sb[1] = emit_qkt(1, 1)
            mults_q1_0 = emit_attention(0, 1)
            qt_sb[2] = emit_qkt(2, 0)
            kt_sb[2] = emit_qkt(2, 1)
            mults_q1_1 = emit_attention(1, 1)
            qt_sb[3] = emit_qkt(3, 0)
            kt_sb[3] = emit_qkt(3, 1)
            mults_q1_2 = emit_attention(2, 1)
            mults_q1_3 = emit_attention(3, 1)
            # projections done: release qkv psum (2 banks) for out-proj
            qkv_ps_ctx.__exit__(None, None, None)
            o_ps_ctx = tc.tile_pool(name="o_ps", bufs=2, space="PSUM")
            o_ps_holder["pool"] = o_ps_ctx.__enter__()

            # qc0 pairs in S/S/A/A form: pair p+1's scores run while pair
            # p's exps trail, so the attnV never waits on a fresh exp.
            mults = {}
            mults[(0, 0)] = emit_attention(0, 0)
            mults[(1, 0)] = emit_attention(1, 0)
            # qc1 normalize mults: r_bc broadcasts have long since landed
            for m in (mults_q1_0, mults_q1_1, mults_q1_2, mults_q1_3):
                m()
            for tsub in range(2):
                for cc in range(2):
                    emit_out_group(1, tsub, cc)
            mults[(2, 0)] = emit_attention(2, 0)
            mults[(3, 0)] = emit_attention(3, 0, pe_bcast=True)
            # out(1) g5-8 here: ~3.4us of PE work with no dependency on
            # pair 3's normalize chain -- hides it completely.
            for tsub in range(2, 4):
                for cc in range(2):
                    emit_out_group(1, tsub, cc)
            for pair in range(PAIRS):
                mults[(pair, 0)]()
            for tsub in range(4):
                for cc in range(2):
                    emit_out_group(0, tsub, cc, last=(tsub == 3 and cc == 1))

            o_ps_ctx.__exit__(None, None, None)
            v_ps_ctx.__exit__(None, None, None)
            s_ps_ctx.__exit__(None, None, None)

    nc.compile()
    return nc


def _host_shards(x, mask, W_in, b_in, W_out, b_out):
    """Build the 8 per-core input maps (bf16, pre-shuffled layouts)."""
    del mask  # causal structure hardcoded (tri_add built locally)
    x = np.asarray(x, dtype=np.float32)
    W_in = np.asarray(W_in, dtype=np.float32)
    b_in = np.asarray(b_in, dtype=np.float32)
    W_out = np.asarray(W_out, dtype=np.float32)

    k = np.arange(P)
    tri_add = np.where(k[:, None] <= k[None, :], 0.0, -1e9).astype(BF)
    ident = np.eye(P, dtype=BF)
    e2 = np.zeros((33, P), dtype=BF)
    e2[0, 0:D] = 1.0
    e2[32, D:P] = 1.0
    xTs = [np.ascontiguousarray(x[b].T.astype(BF)) for b in range(B)]

    per_group = {}
    for g in range(2):
        wqk = np.empty((8, P, KI, P), dtype=BF)
        bqk = np.empty((P, 8), dtype=np.float32)
        wi_r = W_in.reshape(KI, P, 3 * C)
        for p in range(PAIRS):
            qcols = slice((8 * g + 2 * p) * D, (8 * g + 2 * p + 2) * D)
            kcols = slice(C + (8 * g + 2 * p) * D, C + (8 * g + 2 * p + 2) * D)
            # wqk[slot, p, ki, f] = W_in[ki*128+p, cols[f]] (x scale for Q)
            wqk[2 * p] = (wi_r[:, :, qcols] * 0.125).transpose(1, 0, 2).astype(BF)
            wqk[2 * p + 1] = wi_r[:, :, kcols].transpose(1, 0, 2).astype(BF)
            bqk[:, 2 * p] = b_in[qcols] * 0.125
            bqk[:, 2 * p + 1] = b_in[kcols]
        vcols = slice(2 * C + g * 512, 2 * C + (g + 1) * 512)
        wv = np.ascontiguousarray(
            wi_r[:, :, vcols].transpose(1, 0, 2).astype(BF)
        )
        # wout[p, po, c] = W_out[g*512 + po*128 + p, c]
        wo = W_out[g * 512 : (g + 1) * 512, :].reshape(PAIRS, P, C)
        wout = np.ascontiguousarray(wo.transpose(1, 0, 2).astype(BF))
        per_group[g] = dict(
            wqk01=np.ascontiguousarray(wqk[0:2]),
            wqk27=np.ascontiguousarray(wqk[2:8]),
            bqk=bqk, wv=wv, wout=wout,
            tri_add=tri_add, ident=ident, e2=e2,
        )

    in_maps = []
    for c in range(8):
        b, g = c // 2, c % 2
        m = dict(per_group[g])
        m["xT"] = xTs[b]
        in_maps.append(m)
    return in_maps


def run(inputs, trace=False):
    if "nc" not in _CACHE:
        _CACHE["nc"] = _build_nc()
    nc = _CACHE["nc"]
    in_maps = _host_shards(**inputs)
    res = run_bass_kernel_spmd(
        nc, in_maps, core_ids=list(range(8)), trace=trace,
        trace_cores=list(range(8)) if trace else None,
    )
    b_in = np.asarray(inputs["b_in"], dtype=np.float32)
    W_out = np.asarray(inputs["W_out"], dtype=np.float32)
    b_out = np.asarray(inputs["b_out"], dtype=np.float32)
    # V-bias folded into the output bias: b_eff = b_out + b_v @ W_out
    b_eff = b_out + b_in[2 * C :].astype(np.float32) @ W_out
    out = np.empty((B, T, C), dtype=np.float32)
    for b in range(B):
        out[b] = (
            np.asarray(res.results[2 * b]["out"], dtype=np.float32)
            + np.asarray(res.results[2 * b + 1]["out"], dtype=np.float32)
            + b_eff
        )
    return out, res


def kernel(**inputs) -> np.ndarray:
    out, _ = run(inputs, trace=False)
    return out
